# revision 11
# baseline (speedup 1.0000x reference)
"""DeepONet forward on 8 Trainium2 NeuronCores.

Reference computation:
    B = branch_mlp(u_in)   # [2048, 256]
    T = trunk_mlp(ys)      # [16384, 256]
    out = T @ B.T + b0     # [16384, 2048]

Sharding: ys rows (query points) split 8 ways; each core runs the full
branch MLP (replicated -- an on-chip AllGather was measured at ~395 us
for 2 MB on this PJRT path, far slower than just recomputing), its
2048-row trunk shard, and the final [2048, 2048] output block.  Host
concatenates the row blocks.

On-device layout is feature-major (features on SBUF partitions, batch in
the free dimension), so every MLP layer is out.T = W.T @ in.T with the
natural [d_in, d_out] weight as the stationary operand.  Matmuls run as
float32r (tf32-rate, fp32 accumulate in PSUM).

Loop order is weight-stationary: for each output m-tile a 4-bank PSUM
tile [128, 2048] accumulates over k with an inner n-run of 4 matmuls per
weight, drained by one wide ScalarE/VectorE op (tanh+bias fused for
hidden layers).  Phase order: trunk L1-L3 -> branch -> trunk L4 ->
final, so the output stores overlap the back half of the compute.
"""

import sys

if "/opt/trn_rl_repo" not in sys.path:
    sys.path.insert(0, "/opt/trn_rl_repo")

import numpy as np

from concourse import bacc, mybir
from concourse import tile
from concourse.bass_utils import run_bass_kernel_spmd

N_CORES = 8
N_U = 2048          # branch batch (functions)
D_U = 256           # u_in width
N_Y = 16384         # trunk batch (query points)
Q = N_Y // N_CORES  # trunk rows per core
P_LAT = 256         # latent dim p
NT = 512            # moving-operand tile / PSUM bank (4-byte dtypes)
PSN = 2048          # psum tile free size (4 banks)

F32 = mybir.dt.float32
F32R = mybir.dt.float32r
BF16 = mybir.dt.bfloat16
AF = mybir.ActivationFunctionType

BRANCH_DIMS = [256, 512, 512, 512, 256]
TRUNK_DIMS = [2, 512, 512, 512, 256]

MM_DT = F32R  # matmul dtype for weights/activations

_cache = {}


class _Ctx:
    def __init__(self, nc, pool, psum, handles):
        self.nc = nc
        self.pool = pool
        self.psum = psum
        self.handles = handles
        self.drain_flip = 0

    def drain(self, out_ap, ps_ap, bias_ap):
        """PSUM -> SBUF with bias add, alternating ScalarE / VectorE."""
        self.drain_flip ^= 1
        if self.drain_flip:
            self.nc.scalar.activation(out_ap, ps_ap, AF.Identity, bias=bias_ap)
        else:
            self.nc.vector.tensor_scalar_add(out_ap, ps_ap, bias_ap)


def _load_layer(ctx, name, l, d_in, d_out):
    """DMA one layer's weight k-tiles + bias m-tiles into SBUF."""
    nc, pool = ctx.nc, ctx.pool
    n_k = (d_in + 127) // 128
    w_tiles = []
    for k in range(n_k):
        kp = min(128, d_in - k * 128)
        wt = pool.tile([kp, d_out], MM_DT, tag=f"{name}_w{l}_{k}", name=f"{name}_w{l}_{k}")
        nc.sync.dma_start(
            wt[:], ctx.handles[f"{name}w{l}"][k * 128 : k * 128 + kp, :]
        )
        w_tiles.append(wt)
    b_tiles = []
    for m in range(d_out // 128):
        bt = pool.tile([128, 1], F32, tag=f"{name}_b{l}_{m}", name=f"{name}_b{l}_{m}")
        nc.sync.dma_start(
            bt[:], ctx.handles[f"{name}b{l}"][m * 128 : (m + 1) * 128, :]
        )
        b_tiles.append(bt)
    return w_tiles, b_tiles


def _layer(ctx, in_tiles, w_tiles, b_tiles, out_tiles, n_cols, act):
    """One dense layer in feature-major layout: out.T = f(W.T @ in.T + b).

    Weight-stationary: psum holds all n_cols for one m-tile (up to 4
    banks); inner n-run reuses each [128, 128] weight tile 4x.
    """
    nc, psum = ctx.nc, ctx.psum
    for m in range(len(out_tiles)):
        for p2 in range(max(1, n_cols // PSN)):
            pw = min(PSN, n_cols)
            ps = psum.tile([128, pw], F32, tag="ps", name="ps", bufs=2,
                           padded_shape=[128, PSN])
            for k in range(len(in_tiles)):
                for n in range(pw // NT):
                    nsl = slice(p2 * PSN + n * NT, p2 * PSN + (n + 1) * NT)
                    hsl = slice(n * NT, (n + 1) * NT)
                    nc.tensor.matmul(
                        ps[:, hsl],
                        w_tiles[k][:, m * 128 : (m + 1) * 128],
                        in_tiles[k][:, nsl],
                        start=(k == 0),
                        stop=(k == len(in_tiles) - 1),
                    )
            osl = slice(p2 * PSN, p2 * PSN + pw)
            if act:
                nc.scalar.activation(
                    out_tiles[m][:, osl], ps[:], AF.Tanh, bias=b_tiles[m][:]
                )
            else:
                ctx.drain(out_tiles[m][:, osl], ps[:], b_tiles[m][:])


def _emit_body(nc, tc, pool, opool, psum, handles, out_dram):
    ctx = _Ctx(nc, pool, psum, handles)

    def sbt(shape, tag, dt=MM_DT):
        return pool.tile(shape, dt, tag=tag, name=tag)

    # --- input DMAs (trunk inputs first: they unblock PE fastest)
    yt = sbt([TRUNK_DIMS[0], Q], "ysT")
    nc.sync.dma_start(yt[:], handles["ysT"][:, :])
    b0t = pool.tile([128, 1], F32, tag="b0b", name="b0b")
    nc.sync.dma_start(b0t[:], handles["b0b"][:, :])

    def acts(tagbase):
        return [sbt([128, Q], f"{tagbase}_{m}") for m in range(4)]

    bt_ = [sbt([128, N_U], f"bt_{k}") for k in range(2)]

    # --- trunk L1-L3
    w, b = _load_layer(ctx, "t", 0, TRUNK_DIMS[0], TRUNK_DIMS[1])
    t1 = acts("A")
    _layer(ctx, [yt], w, b, t1, Q, act=True)
    w, b = _load_layer(ctx, "t", 1, TRUNK_DIMS[1], TRUNK_DIMS[2])
    t2 = acts("B")
    _layer(ctx, t1, w, b, t2, Q, act=True)
    w, b = _load_layer(ctx, "t", 2, TRUNK_DIMS[2], TRUNK_DIMS[3])
    t3 = acts("A")
    _layer(ctx, t2, w, b, t3, Q, act=True)
    tw3, tb3 = _load_layer(ctx, "t", 3, TRUNK_DIMS[3], TRUNK_DIMS[4])

    # --- branch (u_in.T loads overlap trunk compute)
    u_tiles = []
    for k in range(D_U // 128):
        ut = sbt([128, N_U], f"uT_{k}")
        nc.sync.dma_start(ut[:], handles["u_inT"][k * 128 : (k + 1) * 128, :])
        u_tiles.append(ut)

    w, b = _load_layer(ctx, "b", 0, BRANCH_DIMS[0], BRANCH_DIMS[1])
    b1 = acts("B")
    _layer(ctx, u_tiles, w, b, b1, N_U, act=True)
    w, b = _load_layer(ctx, "b", 1, BRANCH_DIMS[1], BRANCH_DIMS[2])
    b2 = acts("C")
    _layer(ctx, b1, w, b, b2, N_U, act=True)
    w, b = _load_layer(ctx, "b", 2, BRANCH_DIMS[2], BRANCH_DIMS[3])
    b3 = acts("B")
    _layer(ctx, b2, w, b, b3, N_U, act=True)
    w, b = _load_layer(ctx, "b", 3, BRANCH_DIMS[3], BRANCH_DIMS[4])
    _layer(ctx, b3, w, b, bt_, N_U, act=False)

    # --- trunk L4 (output reuses the u_in slots)
    tt = [sbt([128, Q], f"uT_{k}") for k in range(2)]
    _layer(ctx, t3, tw3, tb3, tt, Q, act=False)

    # --- final: out[q, u] = T_c @ B.T + b0
    for mq in range(Q // 128):
        ostage = opool.tile([128, N_U], F32, tag="ostage", name="ostage")
        ps = psum.tile([128, PSN], F32, tag="ps", name="ps", bufs=2)
        for k in range(P_LAT // 128):
            for n in range(N_U // NT):
                nsl = slice(n * NT, (n + 1) * NT)
                nc.tensor.matmul(
                    ps[:, nsl],
                    tt[k][:, mq * 128 : (mq + 1) * 128],
                    bt_[k][:, nsl],
                    start=(k == 0),
                    stop=(k == P_LAT // 128 - 1),
                )
        ctx.drain(ostage[:, :], ps[:], b0t[:])
        nc.sync.dma_start(out_dram[mq * 128 : (mq + 1) * 128, :], ostage[:])


def _build(reps=1, bench=False):
    nc = bacc.Bacc(None, target_bir_lowering=False)

    specs = {"u_inT": ([D_U, N_U], MM_DT), "ysT": ([TRUNK_DIMS[0], Q], MM_DT),
             "b0b": ([128, 1], F32)}
    for l in range(4):
        specs[f"bw{l}"] = ([BRANCH_DIMS[l], BRANCH_DIMS[l + 1]], MM_DT)
        specs[f"bb{l}"] = ([BRANCH_DIMS[l + 1], 1], F32)
        specs[f"tw{l}"] = ([TRUNK_DIMS[l], TRUNK_DIMS[l + 1]], MM_DT)
        specs[f"tb{l}"] = ([TRUNK_DIMS[l + 1], 1], F32)

    handles = {}
    if bench:
        # timing build: everything lives in on-device DRAM so per-call host
        # transfer (and its multi-ms jitter) disappears
        nc.declare_dram_parameter("dummy_in", [128, 128], F32, isOutput=False)
        out_dram = nc.declare_dram_parameter("dummy_out", [128, 128], F32, isOutput=True)
        for name, (shape, dt) in specs.items():
            handles[name] = nc.dram_tensor(name, shape, dt)
        out_dram = nc.dram_tensor("out", [Q, N_U], F32)
    else:
        for name, (shape, dt) in specs.items():
            handles[name] = nc.declare_dram_parameter(name, shape, dt, isOutput=False)
        out_dram = nc.declare_dram_parameter("out", [Q, N_U], F32, isOutput=True)

    with tile.TileContext(nc) as tc:
        with (
            tc.tile_pool(name="sb", bufs=1) as pool,
            tc.tile_pool(name="ost", bufs=3) as opool,
            tc.tile_pool(name="ps", bufs=2, space="PSUM") as psum,
        ):
            if reps > 1:
                with tc.For_i(0, reps, 1):
                    _emit_body(nc, tc, pool, opool, psum, handles, out_dram)
            else:
                _emit_body(nc, tc, pool, opool, psum, handles, out_dram)

    nc.compile()
    return nc


def _get_nc():
    if "nc" not in _cache:
        _cache["nc"] = _build()
    return _cache["nc"]


def _np_mm_dtype():
    if MM_DT in (F32, F32R):
        return np.float32
    import ml_dtypes

    return ml_dtypes.bfloat16


def make_in_maps(u_in, ys, branch_w, branch_b, trunk_w, trunk_b, b0):
    mdt = _np_mm_dtype()
    u_in = np.asarray(u_in, dtype=np.float32)
    ys = np.asarray(ys, dtype=np.float32)
    b0 = float(np.asarray(b0))

    common = {
        "u_inT": np.ascontiguousarray(u_in.T).astype(mdt),
        "b0b": np.full((128, 1), b0, np.float32),
    }
    for l in range(4):
        common[f"bw{l}"] = np.ascontiguousarray(np.asarray(branch_w[l], np.float32)).astype(mdt)
        common[f"bb{l}"] = np.ascontiguousarray(
            np.asarray(branch_b[l], np.float32).reshape(-1, 1)
        )
        common[f"tw{l}"] = np.ascontiguousarray(np.asarray(trunk_w[l], np.float32)).astype(mdt)
        common[f"tb{l}"] = np.ascontiguousarray(
            np.asarray(trunk_b[l], np.float32).reshape(-1, 1)
        )

    in_maps = []
    for c in range(N_CORES):
        m = dict(common)
        m["ysT"] = np.ascontiguousarray(ys[c * Q : (c + 1) * Q].T).astype(mdt)
        in_maps.append(m)
    return in_maps


def kernel(u_in, ys, branch_w, branch_b, trunk_w, trunk_b, b0):
    nc = _get_nc()
    in_maps = make_in_maps(u_in, ys, branch_w, branch_b, trunk_w, trunk_b, b0)
    res = run_bass_kernel_spmd(nc, in_maps, list(range(N_CORES))).results
    return np.concatenate([res[c]["out"] for c in range(N_CORES)], axis=0)
